# revision 11
# baseline (speedup 1.0000x reference)
"""Trainium2 Bass kernel for the KeywordTree hierarchical-softmax loss.

Reference computation (see problem):
    h = outputs[:, 0, :]                      # [B, H] CLS vectors
    e = embedding[path_nodes]                 # [B, L, H] gathered rows
    dots = einsum("blh,bh->bl", e, h)
    terms = log(sigmoid((2*signs-1) * dots) + 1e-7)
    per_sample = sum_l(terms * (l < len_b)) / len_b
    loss = -mean_b(per_sample)

Sharding: data-parallel over batch. Each of the 8 cores gets 8 samples
(= 384 (sample, level) pairs = 3 tiles of 128 partitions) and a replicated
embedding table, gathers its 384 rows with indirect DMA, computes its part
of the loss on-device, and writes one partial scalar. Host sums 8 scalars.
"""

import os
import numpy as np

from concourse import bass, bacc, mybir
import concourse.tile as tile
from concourse.bass_utils import run_bass_kernel_spmd

B, S, H = 64, 512, 768
N_NODES = 200000
L = 48                      # MAX_DEPTH
EPS = 1e-7
M = 8                       # cores
BPC = B // M                # samples per core
R = BPC * L                 # (sample, level) rows per core
P = 128
G = R // P                  # 128-partition tiles per core
F32 = mybir.dt.float32
I32 = mybir.dt.int32


def _build_nc():
    nc = bacc.Bacc("TRN2", target_bir_lowering=False, debug=False)

    emb = nc.dram_tensor("emb", [N_NODES, H], F32, kind="ExternalInput")
    outs = nc.dram_tensor("outs", [BPC, S, H], F32, kind="ExternalInput")
    nodes = nc.dram_tensor("nodes", [P, G], I32, kind="ExternalInput")
    signs = nc.dram_tensor("signs", [P, G], I32, kind="ExternalInput")
    lens = nc.dram_tensor("lens", [BPC], I32, kind="ExternalInput")
    loss = nc.dram_tensor("loss_part", [1, 1], F32, kind="ExternalOutput")

    # Layout constants baked into the NEFF.
    r = np.arange(R)
    sel_np = (r[None, :] // L == np.arange(BPC)[:, None]).astype(np.float32)
    liota_np = np.ascontiguousarray(
        (r % L).astype(np.float32).reshape(G, P).T)        # [P, G]
    sel_c = nc.inline_tensor(sel_np, "sel_const")          # [BPC, R]
    liota_c = nc.inline_tensor(liota_np, "liota_const")    # [P, G]

    with tile.TileContext(nc) as tc:
        from contextlib import ExitStack
        with ExitStack() as ctx:
            sb = ctx.enter_context(tc.tile_pool(name="sb", bufs=1))
            small = ctx.enter_context(tc.tile_pool(name="small", bufs=G))
            epool = ctx.enter_context(tc.tile_pool(name="epool", bufs=G))
            ph = ctx.enter_context(tc.tile_pool(name="ph", bufs=2, space="PSUM"))
            pl = ctx.enter_context(tc.tile_pool(name="pl", bufs=2, space="PSUM"))
            pacc = ctx.enter_context(tc.tile_pool(name="pacc", bufs=1, space="PSUM"))

            nodes_t = sb.tile([P, G], I32)
            nc.sync.dma_start(nodes_t[:], nodes[:])
            signs_t = sb.tile([P, G], I32)
            nc.sync.dma_start(signs_t[:], signs[:])
            liota_t = sb.tile([P, G], F32)
            nc.sync.dma_start(liota_t[:], liota_c[:])
            sel_d = sb.tile([BPC, R], F32)
            nc.sync.dma_start(sel_d[:], sel_c[:])
            lens_t = sb.tile([BPC, 1], I32)
            nc.sync.dma_start(lens_t[:], lens[:, None])
            hcls_d = sb.tile([BPC, H], F32)
            nc.sync.dma_start(hcls_d[:], outs[:, 0, :])

            # PE LoadWeights only supports a single sync wait, so every
            # matmul input must be last-produced by one semaphore source:
            # bounce DMA-landed matmul operands through DVE.
            sel_t = sb.tile([BPC, R], F32)
            nc.vector.tensor_copy(sel_t[:], sel_d[:])
            hcls = sb.tile([BPC, H], F32)
            nc.vector.tensor_copy(hcls[:], hcls_d[:])
            lens_f = sb.tile([BPC, 1], F32)
            nc.vector.tensor_copy(lens_f[:], lens_t[:])
            eps_t = sb.tile([P, 1], F32)
            nc.vector.memset(eps_t[:], EPS)
            ones_t = sb.tile([P, 1], F32)
            nc.vector.memset(ones_t[:], 1.0)

            acc = pacc.tile([1, 1], F32, space="PSUM")

            for g in range(G):
                gs = slice(P * g, P * (g + 1))
                e_g = epool.tile([P, H], F32)
                nc.gpsimd.indirect_dma_start(
                    out=e_g[:],
                    out_offset=None,
                    in_=emb[:],
                    in_offset=bass.IndirectOffsetOnAxis(
                        ap=nodes_t[:, g:g + 1], axis=0),
                )

                # Broadcast each sample's CLS vector / length to its partitions.
                h_g = ph.tile([P, H], F32, space="PSUM")
                nc.tensor.matmul(h_g[:, 0:512], lhsT=sel_t[:, gs],
                                 rhs=hcls[:, 0:512], start=True, stop=True)
                nc.tensor.matmul(h_g[:, 512:H], lhsT=sel_t[:, gs],
                                 rhs=hcls[:, 512:H], start=True, stop=True)
                len_g = pl.tile([P, 1], F32, space="PSUM")
                nc.tensor.matmul(len_g[:], lhsT=sel_t[:, gs],
                                 rhs=lens_f[:], start=True, stop=True)

                # dots = sum_h e * h. The product overwrites e_g in place.
                # (TensorTensorReduce would fuse these but crashes the HW
                # through this compile path.)
                dots_g = small.tile([P, 1], F32, tag="dots")
                nc.vector.tensor_mul(e_g[:], e_g[:], h_g[:])
                nc.vector.reduce_sum(dots_g[:], e_g[:],
                                     axis=mybir.AxisListType.X)

                # terms = log(sigmoid(sign * dots) + eps)
                sign_g = small.tile([P, 1], F32, tag="sign")
                nc.vector.tensor_scalar(
                    sign_g[:], signs_t[:, g:g + 1], 2.0, 1.0,
                    mybir.AluOpType.mult, mybir.AluOpType.subtract)
                sig_g = small.tile([P, 1], F32, tag="sig")
                nc.scalar.activation(
                    sig_g[:], dots_g[:],
                    mybir.ActivationFunctionType.Sigmoid, scale=sign_g[:])
                term_g = small.tile([P, 1], F32, tag="term")
                nc.scalar.activation(
                    term_g[:], sig_g[:],
                    mybir.ActivationFunctionType.Ln, bias=eps_t[:])

                # weights = (l < len_b) / len_b
                mask_g = small.tile([P, 1], F32, tag="mask")
                nc.vector.tensor_tensor(
                    out=mask_g[:], in0=liota_t[:, g:g + 1], in1=len_g[:],
                    op=mybir.AluOpType.is_lt)
                rec_g = small.tile([P, 1], F32, tag="rec")
                nc.vector.reciprocal(rec_g[:], len_g[:])
                wts_g = small.tile([P, 1], F32, tag="wts")
                nc.vector.tensor_mul(wts_g[:], mask_g[:], rec_g[:])

                # partial += sum_p terms * weights  (partition reduction on PE)
                termw_g = small.tile([P, 1], F32, tag="termw")
                nc.vector.tensor_mul(termw_g[:], term_g[:], wts_g[:])
                nc.tensor.matmul(acc[:], lhsT=termw_g[:], rhs=ones_t[:],
                                 start=(g == 0), stop=(g == G - 1))

            out_sb = sb.tile([1, 1], F32)
            nc.scalar.mul(out_sb[:], acc[:], -1.0 / B)
            nc.sync.dma_start(loss[:, :], out_sb[:])

    nc.compile()
    return nc


_NC = None


def _get_nc():
    global _NC
    if _NC is None:
        _NC = _build_nc()
    return _NC


def _make_in_maps(outputs, embedding, path_nodes, path_signs, path_lengths):
    outputs = np.ascontiguousarray(np.asarray(outputs, dtype=np.float32))
    embedding = np.ascontiguousarray(np.asarray(embedding, dtype=np.float32))
    path_nodes = np.asarray(path_nodes, dtype=np.int32)
    path_signs = np.asarray(path_signs, dtype=np.int32)
    path_lengths = np.asarray(path_lengths, dtype=np.int32)

    in_maps = []
    for c in range(M):
        bs = slice(c * BPC, (c + 1) * BPC)
        in_maps.append({
            "emb": embedding,
            "outs": np.ascontiguousarray(outputs[bs]),
            "nodes": np.ascontiguousarray(
                path_nodes[bs].reshape(G, P).T),
            "signs": np.ascontiguousarray(
                path_signs[bs].reshape(G, P).T),
            "lens": np.ascontiguousarray(path_lengths[bs]),
        })
    return in_maps


def kernel_with_results(outputs, embedding, path_nodes, path_signs,
                        path_lengths, trace=False):
    nc = _get_nc()
    in_maps = _make_in_maps(outputs, embedding, path_nodes, path_signs,
                            path_lengths)
    res = run_bass_kernel_spmd(nc, in_maps, core_ids=list(range(M)),
                               trace=trace)
    total = np.float64(0.0)
    for c in range(M):
        total += np.float64(res.results[c]["loss_part"][0, 0])
    return np.float32(total), res


def kernel(outputs, embedding, path_nodes, path_signs, path_lengths):
    out, _ = kernel_with_results(outputs, embedding, path_nodes, path_signs,
                                 path_lengths,
                                 trace=bool(os.environ.get("BASS_TRACE")))
    return out
